# revision 12
# baseline (speedup 1.0000x reference)
"""Trainium2 Bass kernel for nn_Attention_8589935148.

Attention with softmax over the HEAD axis (dim=1), returning (out, p_attn).

Math notes (B=2, H=16, S=2048, D=64):
  scores s[b,h,q,k] = (q . k) / 8;  mask[b,1,q,k] broadcasts over h, so a
  masked (b,q,k) position is masked for ALL 16 heads -> softmax over h of 16
  equal NEG values = uniform 1/16.  Exact reformulation used here:
      E   = exp(s)                      (raw, unmasked scores)
      F   = (E - 1) * mask              (0 at masked positions)
      Z   = (sum_h E - 16) * mask + 16  (= sum_h exp(s) unmasked; 16 masked)
      p   = (F + 1) / Z                 (= softmax_h unmasked; 1/16 masked)
      out = p @ v
  This needs no select/branch ops: two scalar_tensor_tensor instructions per
  head-tile plus one reciprocal per (q,k) plane.

Sharding: batch x query-block. Core c handles batch c//4, query rows
[(c%4)*512, (c%4+1)*512). All 16 heads stay on-core, so the softmax over
heads is purely local — zero collectives.
"""

import sys

sys.path.insert(0, "/opt/trn_rl_repo")

import numpy as np
import ml_dtypes

import concourse.bass as bass
import concourse.tile as tile
from concourse import bacc, mybir
from concourse.bass_utils import run_bass_kernel_spmd
from concourse.masks import make_identity

F32 = mybir.dt.float32
F32R = mybir.dt.float32r
BF16 = mybir.dt.bfloat16
BF16_NP = np.dtype(ml_dtypes.bfloat16)

B, H, S, D = 2, 16, 2048, 64
NCORES = 8
QBLK = S // 4          # 512 query rows per core
QT = 128               # query tile (partition dim)
NQT = QBLK // QT       # 4 q-tiles per core
KC = 1024              # k chunk width processed per softmax round
NKC = S // KC          # 2
KB = 128               # k sub-block for the PV matmul
NKB = KC // KB         # 8
SCALE = 0.125          # 1/sqrt(64)

ALU = mybir.AluOpType
ACTF = mybir.ActivationFunctionType

# feature flags (perf/bisect knobs)
USE_F32R = True         # float32r QK matmul (full PE rate) vs bf16
QK_DT = F32R if USE_F32R else BF16
USE_GPSIMD_STT = False  # offload half the F-step to GpSimd


def build_nc():
    nc = bacc.Bacc("TRN2", target_bir_lowering=False, debug=False,
                   num_devices=NCORES)

    # Inputs (per-core shards, pre-laid-out on host)
    qT = nc.dram_tensor("qT", [H // 2, 128, QBLK], QK_DT, kind="ExternalInput").ap()
    kT = nc.dram_tensor("kT", [H // 2, 128, S], QK_DT, kind="ExternalInput").ap()
    vp = nc.dram_tensor("vp", [S // KB, KB, H, D], BF16, kind="ExternalInput").ap()
    mk = nc.dram_tensor("mk", [NQT, QT, S], BF16, kind="ExternalInput").ap()
    # Outputs
    p_out = nc.dram_tensor("p_out", [H, QBLK, S], F32, kind="ExternalOutput").ap()
    o_out = nc.dram_tensor("o_out", [H, QBLK, D], F32, kind="ExternalOutput").ap()

    with tile.TileContext(nc) as tc:
        with (
            tc.tile_pool(name="const", bufs=1) as const_pool,
            tc.tile_pool(name="kTp", bufs=H // 2) as kT_pool,
            tc.tile_pool(name="qTp", bufs=H // 2) as qT_pool,
            tc.tile_pool(name="vpp", bufs=S // KB) as v_pool,
            tc.tile_pool(name="mkp", bufs=NQT) as m_pool,
            tc.tile_pool(name="Ep", bufs=17) as E_pool,
            tc.tile_pool(name="ztp", bufs=8) as zt_pool,
            tc.tile_pool(name="zp", bufs=2) as z_pool,
            tc.tile_pool(name="rp", bufs=2) as r_pool,
            tc.tile_pool(name="pp", bufs=3) as p_pool,
            tc.tile_pool(name="ptp", bufs=2) as pt_pool,
            tc.tile_pool(name="op", bufs=1) as o_pool,
            tc.tile_pool(name="spsum", bufs=2, space="PSUM") as s_psum,
            tc.tile_pool(name="ptpsum", bufs=2, space="PSUM") as pt_psum,
            tc.tile_pool(name="opsum", bufs=1, space="PSUM") as o_psum,
        ):
            ident = const_pool.tile([128, 128], F32)
            make_identity(nc, ident[:])

            # ---- resident loads --------------------------------------
            kT_sb = []
            for i in range(H // 2):
                t = kT_pool.tile([128, S], QK_DT, tag="kT")
                nc.sync.dma_start(out=t, in_=kT[i])
                kT_sb.append(t)
            qT_sb = []
            for i in range(H // 2):
                t = qT_pool.tile([128, QBLK], QK_DT, tag="qT")
                nc.sync.dma_start(out=t, in_=qT[i])
                qT_sb.append(t)
            v_sb = []
            for i in range(S // KB):
                t = v_pool.tile([KB, H, D], BF16, tag="vp")
                nc.sync.dma_start(out=t, in_=vp[i])
                v_sb.append(t)
            m_sb = []
            for i in range(NQT):
                t = m_pool.tile([QT, S], BF16, tag="mk")
                nc.sync.dma_start(out=t, in_=mk[i])
                m_sb.append(t)

            # ---- main loop -------------------------------------------
            for qt in range(NQT):
                o_ps = o_psum.tile([QT, H, D], F32, tag="ops")
                for kc in range(NKC):
                    # scores + exp for all 16 heads
                    Es = []
                    for h in range(H):
                        pair, half = h // 2, h % 2
                        lo = 64 * half
                        s_ps = s_psum.tile([QT, KC], F32, tag="sps")
                        lhsT = qT_sb[pair][lo:lo + 64, qt * QT:(qt + 1) * QT]
                        for j in range(KC // 512):
                            rhs = kT_sb[pair][lo:lo + 64,
                                              kc * KC + j * 512: kc * KC + (j + 1) * 512]
                            nc.tensor.matmul(
                                s_ps[:, j * 512:(j + 1) * 512],
                                lhsT=lhsT, rhs=rhs,
                                start=True, stop=True,
                            )
                        E_h = E_pool.tile([QT, KC], BF16, tag="E")
                        nc.scalar.activation(E_h, s_ps, ACTF.Exp, scale=SCALE)
                        Es.append(E_h)

                    # Z = sum_h E  (pairwise tree, bf16, fp32 root)
                    Z = z_pool.tile([QT, KC], F32, tag="Z")
                    for sub in range(KC // 512):
                        sl = slice(sub * 512, (sub + 1) * 512)
                        T = [zt_pool.tile([QT, 512], BF16, tag="zt",
                                          name=f"zt{qt}_{kc}_{sub}_{j}")
                             for j in range(8)]
                        for j in range(8):
                            nc.vector.tensor_add(T[j], Es[2 * j][:, sl],
                                                 Es[2 * j + 1][:, sl])
                        for j in range(0, 8, 2):
                            nc.vector.tensor_add(T[j], T[j], T[j + 1])
                        nc.vector.tensor_add(T[0], T[0], T[2])
                        nc.vector.tensor_add(T[4], T[4], T[6])
                        nc.vector.tensor_add(Z[:, sl], T[0], T[4])

                    # Zc = (Z - 16) * mask + 16 ;  R = 1/Zc
                    mt = m_sb[qt][:, kc * KC:(kc + 1) * KC]
                    nc.vector.scalar_tensor_tensor(
                        out=Z, in0=Z, scalar=-16.0, in1=mt,
                        op0=ALU.add, op1=ALU.mult)
                    nc.vector.tensor_scalar_add(Z, Z, 16.0)
                    R = r_pool.tile([QT, KC], F32, tag="R")
                    nc.vector.reciprocal_approx_fast(R, Z)

                    # normalize + transpose + PV per head
                    for h in range(H):
                        E_h = Es[h]
                        # F = (E - 1) * mask, in place (split DVE / GpSimd)
                        f_eng = nc.gpsimd if USE_GPSIMD_STT else nc.vector
                        f_eng.scalar_tensor_tensor(
                            out=E_h[:, 0:512], in0=E_h[:, 0:512], scalar=-1.0,
                            in1=mt[:, 0:512], op0=ALU.add, op1=ALU.mult)
                        nc.vector.scalar_tensor_tensor(
                            out=E_h[:, 512:KC], in0=E_h[:, 512:KC], scalar=-1.0,
                            in1=mt[:, 512:KC], op0=ALU.add, op1=ALU.mult)
                        # P = (F + 1) * R   (fp32, the p_attn payload)
                        P = p_pool.tile([QT, KC], F32, tag="P")
                        nc.vector.scalar_tensor_tensor(
                            out=P, in0=E_h, scalar=1.0, in1=R,
                            op0=ALU.add, op1=ALU.mult)
                        dma_eng = nc.sync if h % 2 == 0 else nc.scalar
                        dma_eng.dma_start(
                            out=p_out[h, qt * QT:(qt + 1) * QT,
                                      kc * KC:(kc + 1) * KC],
                            in_=P)
                        # PT = P^T (PE transpose, fp32 -> bf16 on evict)
                        PT = pt_pool.tile([QT, KC], BF16, tag="PT")
                        for t in range(2):
                            pt_ps = pt_psum.tile([128, 512], F32, tag="pts")
                            for j in range(4):
                                blk = (t * 4 + j) * 128
                                nc.tensor.transpose(
                                    pt_ps[:, j * 128:(j + 1) * 128],
                                    P[:, blk:blk + 128], ident)
                            nc.scalar.copy(PT[:, t * 512:(t + 1) * 512], pt_ps)
                        # out[q,d] += P^T.T @ V
                        for kb in range(NKB):
                            kbg = kc * NKB + kb
                            nc.tensor.matmul(
                                o_ps[:, h, :],
                                lhsT=PT[:, kb * KB:(kb + 1) * KB],
                                rhs=v_sb[kbg][:, h, :],
                                start=(kc == 0 and kb == 0 and h % 8 == 0),
                                stop=(kc == NKC - 1 and kb == NKB - 1
                                      and h % 8 == 7),
                            )

                # evict attention output for this q-tile
                O = o_pool.tile([QT, H, D], F32, tag="O")
                nc.scalar.copy(O, o_ps)
                for h in range(H):
                    nc.sync.dma_start(
                        out=o_out[h, qt * QT:(qt + 1) * QT, :], in_=O[:, h, :])

    nc.compile()
    return nc


_NC_CACHE = None


def _get_nc():
    global _NC_CACHE
    if _NC_CACHE is None:
        _NC_CACHE = build_nc()
    return _NC_CACHE


def _prep_core_inputs(query, key, value, mask):
    """Host-side shard + relayout. Returns list of 8 in_maps."""
    q = np.asarray(query, np.float32)
    k = np.asarray(key, np.float32)
    v = np.asarray(value, np.float32)
    m = np.asarray(mask)

    in_maps = []
    for b in range(B):
        # K^T packed: [16,2048,64] -> [16,64,2048] -> [8,128,2048]
        kTb = np.ascontiguousarray(k[b].transpose(0, 2, 1)).reshape(H // 2, 128, S)
        # V packed: [16,2048,64] -> [2048,16,64] -> [16,128,16,64] bf16
        vpb = np.ascontiguousarray(v[b].transpose(1, 0, 2)).reshape(
            S // KB, KB, H, D).astype(BF16_NP)
        for qi in range(NCORES // B):
            q0 = qi * QBLK
            qTb = np.ascontiguousarray(
                q[b, :, q0:q0 + QBLK, :].transpose(0, 2, 1)).reshape(
                H // 2, 128, QBLK)
            mkb = m[b, 0, q0:q0 + QBLK, :].astype(BF16_NP).reshape(NQT, QT, S)
            in_maps.append({"qT": qTb, "kT": kTb, "vp": vpb, "mk": mkb})
    # core order: b-major then q-block -> core c = b*4 + qi
    return in_maps


def run_on_cores(query, key, value, mask, trace=False, **kw):
    nc = _get_nc()
    in_maps = _prep_core_inputs(query, key, value, mask)
    core_ids = list(range(NCORES))
    res = run_bass_kernel_spmd(nc, in_maps, core_ids, trace=trace, **kw)

    out = np.empty((B, H, S, D), np.float32)
    p_attn = np.empty((B, H, S, S), np.float32)
    for c in range(NCORES):
        b, qi = c // (NCORES // B), c % (NCORES // B)
        q0 = qi * QBLK
        out[b, :, q0:q0 + QBLK, :] = res.results[c]["o_out"]
        p_attn[b, :, q0:q0 + QBLK, :] = res.results[c]["p_out"]
    return (out, p_attn), res


def kernel(query, key, value, mask):
    (out, p_attn), _ = run_on_cores(query, key, value, mask)
    return (out, p_attn)
